# revision 4
# baseline (speedup 1.0000x reference)
"""Megatron-style MHA on 8 Trainium2 NeuronCores.

Problem: B=4, T=2048, C=1024, 16 heads, head_dim=64, causal attention, fp32.
  qkv = x @ Wqkv^T; attention per head; out = attn @ Wproj^T

Sharding (tensor-parallel over heads + AllToAll reshard):
  - Core c owns heads {2c, 2c+1}: computes Q/K/V (column-parallel Wqkv slice)
    and causal attention for those heads over all batches/positions.
  - Attention outputs (kept transposed: [feature, t]) are resharded with two
    AllToAll collectives (one for batches 0-1, one for 2-3) so that each core
    ends up with the full 1024 attn features for 1/8 of the t positions.
  - Each core then applies the full Wproj to its t-slice (data-parallel), so
    no reduction collective is needed.

All matmuls run in float32r (fp32 stored, E8M11-rounded inputs, fp32
accumulate) which streams at full PE rate for moving dims >= 256.

Everything on-device is laid out "transposed" ([feature, t]) so that the
contraction dim of every matmul lands on SBUF partitions and no transposes
are needed anywhere except V (done on the PE with an identity matmul).

Softmax: scores are O(1) (inputs are unit-scale gaussians), so exp() without
max-subtraction is safe in fp32. The softmax denominator is produced by the
same matmul that computes attn@V via a ones-column appended to V; the final
divide is a reciprocal + a rank-1 broadcast matmul + an elementwise multiply.
"""

import numpy as np

import concourse.bass as bass
import concourse.mybir as mybir
import concourse.tile as tile
from concourse import bacc
from concourse.bass_utils import run_bass_kernel_spmd

B, T, C, H, D = 4, 2048, 1024, 16, 64
NCORE = 8
HPC = H // NCORE  # 2 heads per core
BT = B * T
TCH = 512  # t-chunk width for qkv / scores free dim
NKT = T // 128  # 16 k-tiles per batch
NQC = T // TCH  # 4 q-chunks per batch

F32 = mybir.dt.float32
F32R = mybir.dt.float32r
EXP = mybir.ActivationFunctionType.Exp
MULT = mybir.AluOpType.mult


def round_fp32r(a: np.ndarray) -> np.ndarray:
    """Round fp32 to E8M11 (fp32r) with round-to-nearest-even, as the HW does."""
    u = np.ascontiguousarray(a, dtype=np.float32).view(np.uint32)
    lsb = (u >> 12) & 1
    r = (u + 0x7FF + lsb) & 0xFFFFF000
    return r.view(np.float32)


def build_nc():
    nc = bacc.Bacc("TRN2", target_bir_lowering=False, debug=False, num_devices=NCORE)

    xT = nc.dram_tensor("xT", [C, BT], F32R, kind="ExternalInput")
    wqkvT = nc.dram_tensor("wqkvT", [C, 3 * 128], F32R, kind="ExternalInput")
    wprojT = nc.dram_tensor("wprojT", [C, C], F32R, kind="ExternalInput")
    ident = nc.dram_tensor("ident", [128, 128], F32, kind="ExternalInput")
    tri = nc.dram_tensor("tri", [128, 128], F32R, kind="ExternalInput")
    ones64 = nc.dram_tensor("ones64", [1, 64], F32R, kind="ExternalInput")
    yT = nc.dram_tensor("yT", [C, 2 * TCH], F32, kind="ExternalOutput")

    # AllToAll buffers: [8 chunks, 128 feat (2 heads), 512 t]
    a2a_in = [
        nc.dram_tensor(f"a2a_in{i}", [NCORE, 128, TCH], F32R, kind="Internal")
        for i in range(2)
    ]
    a2a_out = [
        nc.dram_tensor(f"a2a_out{i}", [NCORE, 128, TCH], F32R, kind="Internal")
        for i in range(2)
    ]
    groups = [list(range(NCORE))]

    with tile.TileContext(nc) as tc:
        with (
            tc.tile_pool(name="const", bufs=1) as constp,
            tc.tile_pool(name="xt", bufs=10) as xtp,
            tc.tile_pool(name="kt", bufs=2) as ktp,
            tc.tile_pool(name="qt", bufs=2) as qtp,
            tc.tile_pool(name="vaug", bufs=2) as vaugp,
            tc.tile_pool(name="vstage", bufs=3) as vstagep,
            tc.tile_pool(name="pt", bufs=4) as ptp,
            tc.tile_pool(name="ostage", bufs=3) as ostagep,
            tc.tile_pool(name="rec", bufs=2) as recp,
            tc.tile_pool(name="ofin", bufs=2) as ofinp,
            tc.tile_pool(name="recv", bufs=10) as recvp,
            tc.tile_pool(name="ystage", bufs=3) as ystagep,
            tc.tile_pool(name="psq", bufs=2, space="PSUM") as psq,
            tc.tile_pool(name="pss", bufs=2, space="PSUM") as pss,
            tc.tile_pool(name="pso", bufs=2, space="PSUM") as pso,
        ):
            # ---- constants ----
            wqkv_sb = constp.tile([128, C // 128, 3 * 128], F32R, tag="wqkv")
            nc.sync.dma_start(
                wqkv_sb[:], wqkvT[:].rearrange("(ct p) o -> p ct o", p=128)
            )
            wproj_sb = constp.tile([128, C // 128, C], F32R, tag="wproj")
            nc.sync.dma_start(
                wproj_sb[:], wprojT[:].rearrange("(ct p) o -> p ct o", p=128)
            )
            ident_sb = constp.tile([128, 128], F32, tag="ident")
            nc.sync.dma_start(ident_sb[:], ident[:])
            tri_sb = constp.tile([128, 128], F32R, tag="tri")
            nc.sync.dma_start(tri_sb[:], tri[:])
            ones_sb = constp.tile([1, 64], F32R, tag="ones")
            nc.sync.dma_start(ones_sb[:], ones64[:])

            # Pre-zero score PSUM slots: diagonal tiles only write the causal
            # column range, and exp() reads the full (paired) range; stale
            # bits from uninitialized PSUM could be NaN/Inf otherwise.
            for _ in range(2):
                z = pss.tile([128, 2 * TCH], F32, tag="s")
                nc.vector.memset(z[:], 0.0)

            def qkv_batch(b):
                """Q^T,K^T: [128 (2 heads x 64d), 2048] f32r. V -> vaug tiles."""
                kt_t = ktp.tile([128, T], F32R, tag="kt")
                qt_t = qtp.tile([128, T], F32R, tag="qt")
                va_t = vaugp.tile([128, NKT, 130], F32R, tag="vaug")
                # ones columns at 64 and 129 of each [*, kt, :] slice: fill the
                # whole tile with 1.0; the V copies overwrite cols 0:64, 65:129
                nc.vector.memset(va_t[:].bitcast(F32), 1.0)
                for tch in range(T // TCH):
                    t0 = b * T + tch * TCH
                    xts = []
                    for ct in range(C // 128):
                        xt_tile = xtp.tile([128, TCH], F32R, tag="xt")
                        nc.sync.dma_start(
                            xt_tile[:], xT[ct * 128 : (ct + 1) * 128, t0 : t0 + TCH]
                        )
                        xts.append(xt_tile)
                    for o in range(3):  # q, k, v feature blocks (128 each)
                        ps = psq.tile([128, TCH], F32, tag="q")
                        for ct in range(C // 128):
                            nc.tensor.matmul(
                                ps[:],
                                wqkv_sb[:, ct, o * 128 : (o + 1) * 128],
                                xts[ct][:],
                                start=(ct == 0),
                                stop=(ct == C // 128 - 1),
                            )
                        sl = slice(tch * TCH, (tch + 1) * TCH)
                        if o == 0:
                            nc.vector.tensor_copy(qt_t[:, sl], ps[:])
                        elif o == 1:
                            nc.vector.tensor_copy(kt_t[:, sl], ps[:])
                        else:
                            vs = vstagep.tile([128, TCH], F32, tag="vs")
                            nc.vector.tensor_copy(vs[:], ps[:])
                            for tt in range(TCH // 128):
                                kti = tch * (TCH // 128) + tt
                                psv = psq.tile([128, 128], F32, tag="q")
                                nc.tensor.transpose(
                                    psv[:],
                                    vs[:, tt * 128 : (tt + 1) * 128],
                                    ident_sb[:],
                                )
                                # [128 t, 128 d2] -> vaug cols {0:64, 65:129}
                                dst = va_t[:, kti].rearrange(
                                    "p (two s) -> p two s", s=65
                                )[:, :, 0:64]
                                nc.vector.tensor_copy(
                                    dst, psv[:].rearrange("p (two s) -> p two s", s=64)
                                )
                return qt_t, kt_t, va_t

            def attn_batch(b, qt_t, kt_t, va_t):
                for hl in range(HPC):
                    ost = ostagep.tile([65, T], F32, tag="ost")
                    ofin = ofinp.tile([64, T], F32R, tag="ofin")
                    for qc in range(NQC):
                        ktmax = (qc + 1) * (TCH // 128)
                        psO = pso.tile([65, TCH], F32, tag="o")
                        for ktp_i in range(ktmax // 2):
                            kts = [2 * ktp_i, 2 * ktp_i + 1]
                            psS = pss.tile([128, 2 * TCH], F32, tag="s")
                            pt = ptp.tile([128, 2 * TCH], F32R, tag="pt")
                            colLo = []
                            for i, kt in enumerate(kts):
                                lo = max(0, 128 * kt - TCH * qc)
                                colLo.append(lo)
                                nc.tensor.matmul(
                                    psS[:, TCH * i + lo : TCH * (i + 1)],
                                    kt_t[64 * hl : 64 * hl + 64,
                                         128 * kt : 128 * (kt + 1)],
                                    qt_t[64 * hl : 64 * hl + 64,
                                         TCH * qc + lo : TCH * (qc + 1)],
                                    start=True,
                                    stop=True,
                                )
                            if colLo[0] == 0 and colLo[1] == 0:
                                nc.scalar.activation(
                                    pt[:], psS[:], EXP, scale=0.125
                                )
                            else:
                                for i, lo in enumerate(colLo):
                                    nc.scalar.activation(
                                        pt[:, TCH * i + lo : TCH * (i + 1)],
                                        psS[:, TCH * i + lo : TCH * (i + 1)],
                                        EXP,
                                        scale=0.125,
                                    )
                            for i, kt in enumerate(kts):
                                lo = colLo[i]
                                if kt >= 4 * qc:  # diagonal: mask boundary block
                                    nc.vector.tensor_mul(
                                        pt[:, TCH * i + lo : TCH * i + lo + 128],
                                        pt[:, TCH * i + lo : TCH * i + lo + 128],
                                        tri_sb[:],
                                    )
                            for i, kt in enumerate(kts):
                                lo = colLo[i]
                                nc.tensor.matmul(
                                    psO[:, lo:TCH],
                                    va_t[:, kt, 65 * hl : 65 * (hl + 1)],
                                    pt[:, TCH * i + lo : TCH * (i + 1)],
                                    start=(kt == 0),
                                    stop=(kt == ktmax - 1),
                                )
                        nc.vector.tensor_copy(
                            ost[:, TCH * qc : TCH * (qc + 1)], psO[:]
                        )
                    # normalize: out = O' / denom  (denom = ost row 64)
                    for qc in range(NQC):
                        sl = slice(TCH * qc, TCH * (qc + 1))
                        rec = recp.tile([1, TCH], F32R, tag="rec")
                        with nc.allow_low_precision("fp32r softmax denominators"):
                            nc.vector.reciprocal(rec[:], ost[64:65, sl])
                        psB = psq.tile([64, TCH], F32, tag="q")
                        nc.tensor.matmul(
                            psB[:], ones_sb[:], rec[:], start=True, stop=True
                        )
                        nc.vector.tensor_mul(ofin[:, sl], ost[0:64, sl], psB[:])
                        # ship to the AllToAll send buffer
                        half = b // 2
                        j = 4 * (b % 2) + qc
                        nc.sync.dma_start(
                            a2a_in[half][j, 64 * hl : 64 * hl + 64, :],
                            ofin[:, sl],
                        )

            def proj_half(half):
                recvs = []
                for ct in range(C // 128):
                    r = recvp.tile([128, TCH], F32R, tag="recv")
                    nc.sync.dma_start(r[:], a2a_out[half][ct])
                    recvs.append(r)
                for o in range(C // 128):
                    psY = psq.tile([128, TCH], F32, tag="q")
                    for ct in range(C // 128):
                        nc.tensor.matmul(
                            psY[:],
                            wproj_sb[:, ct, o * 128 : (o + 1) * 128],
                            recvs[ct][:],
                            start=(ct == 0),
                            stop=(ct == C // 128 - 1),
                        )
                    ys = ystagep.tile([128, TCH], F32, tag="ys")
                    nc.vector.tensor_copy(ys[:], psY[:])
                    nc.sync.dma_start(
                        yT[o * 128 : (o + 1) * 128, half * TCH : (half + 1) * TCH],
                        ys[:],
                    )

            def a2a(half):
                nc.gpsimd.collective_compute(
                    "AllToAll",
                    mybir.AluOpType.bypass,
                    replica_groups=groups,
                    ins=[a2a_in[half][:]],
                    outs=[a2a_out[half][:]],
                )

            for b in range(B):
                tiles = qkv_batch(b)
                attn_batch(b, *tiles)
                if b == 1:
                    a2a(0)
                if b == 2:
                    proj_half(0)
                if b == 3:
                    a2a(1)
                    proj_half(1)

    nc.compile()
    return nc


_NC_CACHE = None


def kernel(x: np.ndarray, Wqkv: np.ndarray, Wproj: np.ndarray) -> np.ndarray:
    global _NC_CACHE
    x = np.asarray(x, dtype=np.float32)
    Wqkv = np.asarray(Wqkv, dtype=np.float32)
    Wproj = np.asarray(Wproj, dtype=np.float32)

    xT = round_fp32r(x.reshape(BT, C).T)
    wprojT = round_fp32r(Wproj.T)
    ident = np.eye(128, dtype=np.float32)
    r = np.arange(128)
    tri = (r[:, None] <= r[None, :]).astype(np.float32)  # valid iff row <= col
    ones64 = np.ones((1, 64), dtype=np.float32)

    in_maps = []
    for c in range(NCORE):
        rows = slice(c * HPC * D, (c + 1) * HPC * D)  # 128 feature rows
        wq = Wqkv[0 * C :][rows]
        wk = Wqkv[1 * C :][rows]
        wv = Wqkv[2 * C :][rows]
        wqkvT_c = round_fp32r(np.concatenate([wq, wk, wv], axis=0).T)
        in_maps.append(
            {
                "xT": xT,
                "wqkvT": wqkvT_c,
                "wprojT": wprojT,
                "ident": ident,
                "tri": tri,
                "ones64": ones64,
            }
        )

    if _NC_CACHE is None:
        _NC_CACHE = build_nc()
    res = run_bass_kernel_spmd(_NC_CACHE, in_maps, core_ids=list(range(NCORE)))

    # reassemble: core j returned yT_j [1024, 1024] =
    #   [t of batches 0-1, slice j (512) | t of batches 2-3, slice j (512)]
    yT = np.empty((C, BT), dtype=np.float32)
    for j, r_ in enumerate(res.results):
        yTj = r_["yT"]
        yT[:, TCH * j : TCH * (j + 1)] = yTj[:, :TCH]
        yT[:, BT // 2 + TCH * j : BT // 2 + TCH * (j + 1)] = yTj[:, TCH:]
    return np.ascontiguousarray(yT.T).reshape(B, T, C)


# revision 31
# speedup vs baseline: 118.0668x; 118.0668x over previous
"""Megatron-style MHA on 8 Trainium2 NeuronCores.

Problem: B=4, T=2048, C=1024, 16 heads, head_dim=64, causal attention, fp32.
  qkv = x @ Wqkv^T; attention per head; out = attn @ Wproj^T

Sharding (tensor-parallel over heads + AllToAll reshard):
  - Core c owns heads {2c, 2c+1}: computes Q/K/V (column-parallel Wqkv slice)
    and causal attention for those heads over all batches/positions.
  - Attention outputs (kept transposed: [feature, t]) are resharded with two
    AllToAll collectives (one for batches 0-1, one for 2-3) so that each core
    ends up with the full 1024 attn features for 1/8 of the t positions.
  - Each core then applies the full Wproj to its t-slice (data-parallel), so
    no reduction collective is needed.

All matmuls run in float32r (fp32 stored, E8M11-rounded inputs, fp32
accumulate) which streams at full PE rate for moving dims >= 256.

Everything on-device is laid out "transposed" ([feature, t]) so that the
contraction dim of every matmul lands on SBUF partitions and no transposes
are needed anywhere except V (done on the PE with an identity matmul).

Softmax: scores are O(1) (inputs are unit-scale gaussians), so exp() without
max-subtraction is safe in fp32. The softmax denominator is produced by the
same matmul that computes attn@V via a ones-column appended to V; the final
divide is a reciprocal + a rank-1 broadcast matmul + an elementwise multiply.
"""

import numpy as np

import concourse.bass as bass
import concourse.mybir as mybir
import concourse.tile as tile
from concourse import bacc
from concourse.bass_utils import run_bass_kernel_spmd

B, T, C, H, D = 4, 2048, 1024, 16, 64
NCORE = 8
HPC = H // NCORE  # 2 heads per core
BT = B * T
TCH = 512  # t-chunk width for qkv / scores free dim
NKT = T // 128  # 16 k-tiles per batch
NQC = T // TCH  # 4 q-chunks per batch

F32 = mybir.dt.float32
F32R = mybir.dt.float32r
EXP = mybir.ActivationFunctionType.Exp
MULT = mybir.AluOpType.mult


def round_fp32r(a: np.ndarray) -> np.ndarray:
    """Round fp32 to E8M11 (fp32r) with round-to-nearest-even, as the HW does."""
    u = np.ascontiguousarray(a, dtype=np.float32).view(np.uint32)
    lsb = (u >> 12) & 1
    r = (u + 0x7FF + lsb) & 0xFFFFF000
    return r.view(np.float32)


def build_nc(sim_mode: bool = False, max_stage: int = 99):
    # sim_mode: skip collectives (TimelineSim is single-core) — timing study only
    # max_stage: emit only the first N stages (timing bisection in sim_mode)
    nc = bacc.Bacc("TRN2", target_bir_lowering=False, debug=False, num_devices=NCORE)

    xT = nc.dram_tensor("xT", [C, BT], F32R, kind="ExternalInput")
    wqkvT = nc.dram_tensor("wqkvT", [C, 3 * 128], F32R, kind="ExternalInput")
    wprojT = nc.dram_tensor("wprojT", [C, C], F32R, kind="ExternalInput")
    ident = nc.dram_tensor("ident", [128, 128], F32, kind="ExternalInput")
    tri = nc.dram_tensor("tri", [128, 128], F32R, kind="ExternalInput")
    ones64 = nc.dram_tensor("ones64", [1, 64], F32R, kind="ExternalInput")
    yT = nc.dram_tensor("yT", [C, 2 * TCH], F32, kind="ExternalOutput")

    # AllToAll buffers, one per batch: [8 chunks, 128 feat (2 heads), 256 t]
    QW = T // NCORE  # 256: per-core t-slice of one batch
    a2a_in = [
        nc.dram_tensor(f"a2a_in{i}", [NCORE, 128, QW], F32R, kind="Internal")
        for i in range(B)
    ]
    a2a_out = [
        nc.dram_tensor(f"a2a_out{i}", [NCORE, 128, QW], F32R, kind="Internal")
        for i in range(B)
    ]
    groups = [list(range(NCORE))]

    with tile.TileContext(nc) as tc:
        with (
            tc.tile_pool(name="const", bufs=1) as constp,
            tc.tile_pool(name="xt", bufs=12) as xtp,
            tc.tile_pool(name="kt", bufs=2) as ktp,
            tc.tile_pool(name="qt", bufs=2) as qtp,
            tc.tile_pool(name="vaug", bufs=2) as vaugp,
            tc.tile_pool(name="vstage", bufs=3) as vstagep,
            tc.tile_pool(name="pt", bufs=4) as ptp,
            tc.tile_pool(name="rec", bufs=2) as recp,
            tc.tile_pool(name="bcast", bufs=2) as bcastp,
            tc.tile_pool(name="ofin", bufs=2) as ofinp,
            tc.tile_pool(name="recv", bufs=10) as recvp,
            tc.tile_pool(name="ystage", bufs=2) as ystagep,
            tc.tile_pool(name="psq", bufs=2, space="PSUM") as psq,
            tc.tile_pool(name="pss", bufs=2, space="PSUM") as pss,
            tc.tile_pool(name="pso", bufs=2, space="PSUM") as pso,
        ):
            # ---- constants ----
            # wqkv loads are interleaved with the first x chunk (see qkv_batch)
            wqkv_sb = constp.tile([128, C // 128, 3 * 128], F32R, tag="wqkv")
            wproj_sb = constp.tile([128, C // 128, C], F32R, tag="wproj")

            def load_wproj():
                # deferred: wproj is only needed by proj_half(0), far into the
                # kernel — keep it off the startup DMA critical path
                for ct in range(C // 128):
                    nc.sync.dma_start(
                        wproj_sb[:, ct], wprojT[ct * 128 : (ct + 1) * 128, :]
                    )
            ident_sb = constp.tile([128, 128], F32, tag="ident")
            nc.sync.dma_start(ident_sb[:], ident[:])
            tri_sb = constp.tile([128, 128], F32R, tag="tri")
            nc.sync.dma_start(tri_sb[:], tri[:])
            ones_sb = constp.tile([1, 64], F32R, tag="ones")
            nc.sync.dma_start(ones_sb[:], ones64[:])

            # Pre-zero score PSUM slots: diagonal tiles only write the causal
            # column range, and exp() reads the full (paired) range; stale
            # bits from uninitialized PSUM could be NaN/Inf otherwise.
            for _ in range(2):
                z = pss.tile([128, 2 * TCH], F32, tag="s")
                nc.vector.memset(z[:], 0.0)

            def qkv_batch(b):
                """Q^T,K^T: [128 (2 heads x 64d), 2048] f32r. V -> vaug tiles."""
                kt_t = ktp.tile([128, T], F32R, tag="kt")
                qt_t = qtp.tile([128, T], F32R, tag="qt")
                va_t = vaugp.tile([128, NKT, 130], F32R, tag="vaug")
                # ones columns at 64 and 129 of each [*, kt, :] slice: fill the
                # whole tile with 1.0; the V copies overwrite cols 0:64, 65:129
                nc.gpsimd.memset(va_t[:].bitcast(F32), 1.0)
                for tch in range(T // TCH):
                    t0 = b * T + tch * TCH
                    xts = []
                    for ct in range(C // 128):
                        if b == 0 and tch == 0:
                            # interleave weight-tile loads with the first x
                            # chunk so the first matmul chain starts early
                            nc.sync.dma_start(
                                wqkv_sb[:, ct], wqkvT[ct * 128 : (ct + 1) * 128, :]
                            )
                        xt_tile = xtp.tile([128, TCH], F32R, tag="xt")
                        nc.sync.dma_start(
                            xt_tile[:], xT[ct * 128 : (ct + 1) * 128, t0 : t0 + TCH]
                        )
                        xts.append(xt_tile)
                    for o in range(3):  # q, k, v feature blocks (128 each)
                        ps = psq.tile([128, TCH], F32, tag="q")
                        for ct in range(C // 128):
                            nc.tensor.matmul(
                                ps[:],
                                wqkv_sb[:, ct, o * 128 : (o + 1) * 128],
                                xts[ct][:],
                                start=(ct == 0),
                                stop=(ct == C // 128 - 1),
                            )
                        sl = slice(tch * TCH, (tch + 1) * TCH)
                        if o == 0:
                            nc.vector.tensor_copy(qt_t[:, sl], ps[:])
                        elif o == 1:
                            nc.vector.tensor_copy(kt_t[:, sl], ps[:])
                        else:
                            vs = vstagep.tile([128, TCH], F32, tag="vs")
                            nc.vector.tensor_copy(vs[:], ps[:])
                            for tt in range(TCH // 128):
                                kti = tch * (TCH // 128) + tt
                                psv = psq.tile([128, 128], F32, tag="q")
                                nc.tensor.transpose(
                                    psv[:],
                                    vs[:, tt * 128 : (tt + 1) * 128],
                                    ident_sb[:],
                                )
                                # [128 t, 128 d2] -> vaug cols {0:64, 65:129}
                                dst = va_t[:, kti].rearrange(
                                    "p (two s) -> p two s", s=65
                                )[:, :, 0:64]
                                nc.vector.tensor_copy(
                                    dst, psv[:].rearrange("p (two s) -> p two s", s=64)
                                )
                return qt_t, kt_t, va_t

            def attn_batch(b, qt_t, kt_t, va_t):
                ofin = [ofinp.tile([64, T], F32R, tag="ofin", name=f"ofin{hl}") for hl in range(HPC)]
                for qc in range(NQC):
                    ktmax = (qc + 1) * (TCH // 128)
                    psO = [pso.tile([65, TCH], F32, tag="o", name=f"psO{hl}") for hl in range(HPC)]
                    for ktp_i in range(ktmax // 2):
                        kts = [2 * ktp_i, 2 * ktp_i + 1]
                        colLo = [max(0, 128 * kt - TCH * qc) for kt in kts]
                        psS = [pss.tile([128, 2 * TCH], F32, tag="s", name=f"psS{hl}")
                               for hl in range(HPC)]
                        pt = [ptp.tile([128, 2 * TCH], F32R, tag="pt", name=f"pt{hl}")
                              for hl in range(HPC)]
                        # scores: the two heads' K=64 matmuls go to disjoint
                        # PE row groups (base partitions 0 / 64) and overlap
                        for i, kt in enumerate(kts):
                            for hl in range(HPC):
                                nc.tensor.matmul(
                                    psS[hl][:, TCH * i + colLo[i] : TCH * (i + 1)],
                                    kt_t[64 * hl : 64 * hl + 64,
                                         128 * kt : 128 * (kt + 1)],
                                    qt_t[64 * hl : 64 * hl + 64,
                                         TCH * qc + colLo[i] : TCH * (qc + 1)],
                                    start=True,
                                    stop=True,
                                )
                        for hl in range(HPC):
                            if colLo[0] == 0 and colLo[1] == 0:
                                nc.scalar.activation(
                                    pt[hl][:], psS[hl][:], EXP, scale=0.125
                                )
                            else:
                                # one strided op covering both halves from the
                                # smaller colLo; the extra columns in the
                                # second half are unused downstream
                                lo = min(colLo)
                                src = psS[hl][:].rearrange(
                                    "p (two x) -> p two x", two=2
                                )[:, :, lo:TCH]
                                dst = pt[hl][:].rearrange(
                                    "p (two x) -> p two x", two=2
                                )[:, :, lo:TCH]
                                nc.scalar.activation(dst, src, EXP, scale=0.125)
                        for hl in range(HPC):
                            for i, kt in enumerate(kts):
                                lo = colLo[i]
                                if kt >= 4 * qc:  # diagonal: mask boundary block
                                    nc.gpsimd.tensor_mul(
                                        pt[hl][:, TCH * i + lo : TCH * i + lo + 128],
                                        pt[hl][:, TCH * i + lo : TCH * i + lo + 128],
                                        tri_sb[:],
                                    )
                        for i, kt in enumerate(kts):
                            for hl in range(HPC):
                                nc.tensor.matmul(
                                    psO[hl][:, colLo[i] : TCH],
                                    va_t[:, kt, 65 * hl : 65 * (hl + 1)],
                                    pt[hl][:, TCH * i + colLo[i] : TCH * (i + 1)],
                                    start=(kt == 0),
                                    stop=(kt == ktmax - 1),
                                )
                    # normalize straight out of PSUM: denominator is psO row 64
                    for hl in range(HPC):
                        sl = slice(TCH * qc, TCH * (qc + 1))
                        rec = recp.tile([1, TCH], F32R, tag="rec")
                        with nc.allow_low_precision("fp32r softmax denominators"):
                            nc.vector.reciprocal(rec[:], psO[hl][64:65, :])
                        bc = bcastp.tile([64, TCH], F32R, tag="bc", name=f"bc{hl}")
                        nc.gpsimd.partition_broadcast(bc[:], rec[:])
                        nc.vector.tensor_mul(ofin[hl][:, sl], psO[hl][0:64, :], bc[:])
                        # ship to the AllToAll send buffers (2 chunks per qc)
                        for half in range(2):
                            j = 2 * qc + half
                            nc.sync.dma_start(
                                a2a_in[b][j, 64 * hl : 64 * hl + 64, :],
                                ofin[hl][:, TCH * qc + QW * half :
                                          TCH * qc + QW * (half + 1)],
                            )

            def proj_quarter(b):
                recvs = []
                for ct in range(C // 128):
                    r = recvp.tile([128, QW], F32R, tag="recv")
                    nc.sync.dma_start(r[:], a2a_out[b][ct])
                    recvs.append(r)
                for o in range(C // 128):
                    psY = pso.tile([128, QW], F32, tag="o", name=f"psY{o}")
                    for ct in range(C // 128):
                        nc.tensor.matmul(
                            psY[:],
                            wproj_sb[:, ct, o * 128 : (o + 1) * 128],
                            recvs[ct][:],
                            start=(ct == 0),
                            stop=(ct == C // 128 - 1),
                        )
                    ys = ystagep.tile([128, QW], F32, tag="ys")
                    nc.vector.tensor_copy(ys[:], psY[:])
                    nc.sync.dma_start(
                        yT[o * 128 : (o + 1) * 128, QW * b : QW * (b + 1)],
                        ys[:],
                    )

            def a2a(b):
                if sim_mode:
                    return
                nc.gpsimd.collective_compute(
                    "AllToAll",
                    mybir.AluOpType.bypass,
                    replica_groups=groups,
                    ins=[a2a_in[b][:]],
                    outs=[a2a_out[b][:]],
                )

            stage = 0
            for b in range(B):
                if stage >= max_stage:
                    break
                stage += 1
                tiles = qkv_batch(b)
                if stage >= max_stage:
                    break
                stage += 1
                attn_batch(b, *tiles)
                a2a(b)
                if b == 0:
                    load_wproj()
                if stage < max_stage:
                    stage += 1
                    proj_quarter(b)

    nc.compile()
    return nc


_NC_CACHE = None


def kernel(x: np.ndarray, Wqkv: np.ndarray, Wproj: np.ndarray) -> np.ndarray:
    global _NC_CACHE
    x = np.asarray(x, dtype=np.float32)
    Wqkv = np.asarray(Wqkv, dtype=np.float32)
    Wproj = np.asarray(Wproj, dtype=np.float32)

    xT = round_fp32r(x.reshape(BT, C).T)
    wprojT = round_fp32r(Wproj.T)
    ident = np.eye(128, dtype=np.float32)
    r = np.arange(128)
    tri = (r[:, None] <= r[None, :]).astype(np.float32)  # valid iff row <= col
    ones64 = np.ones((1, 64), dtype=np.float32)

    in_maps = []
    for c in range(NCORE):
        rows = slice(c * HPC * D, (c + 1) * HPC * D)  # 128 feature rows
        wq = Wqkv[0 * C :][rows]
        wk = Wqkv[1 * C :][rows]
        wv = Wqkv[2 * C :][rows]
        wqkvT_c = round_fp32r(np.concatenate([wq, wk, wv], axis=0).T)
        in_maps.append(
            {
                "xT": xT,
                "wqkvT": wqkvT_c,
                "wprojT": wprojT,
                "ident": ident,
                "tri": tri,
                "ones64": ones64,
            }
        )

    if _NC_CACHE is None:
        _NC_CACHE = build_nc()
    res = run_bass_kernel_spmd(_NC_CACHE, in_maps, core_ids=list(range(NCORE)))

    # reassemble: core j returned yT_j [1024, 4*256]; quarter b holds the
    # t-slice [2048*b + 256*j, 2048*b + 256*(j+1)) of the full output
    QW = T // NCORE
    yT = np.empty((C, BT), dtype=np.float32)
    for j, r_ in enumerate(res.results):
        yTj = r_["yT"]
        for b in range(B):
            yT[:, T * b + QW * j : T * b + QW * (j + 1)] = (
                yTj[:, QW * b : QW * (b + 1)]
            )
    return np.ascontiguousarray(yT.T).reshape(B, T, C)


# revision 36
# speedup vs baseline: 120.0675x; 1.0169x over previous
"""Megatron-style MHA on 8 Trainium2 NeuronCores.

Problem: B=4, T=2048, C=1024, 16 heads, head_dim=64, causal attention, fp32.
  qkv = x @ Wqkv^T; attention per head; out = attn @ Wproj^T

Sharding (tensor-parallel over heads + AllToAll reshard):
  - Core c owns heads {2c, 2c+1}: computes Q/K/V (column-parallel Wqkv slice)
    and causal attention for those heads over all batches/positions.
  - Attention outputs (kept transposed: [feature, t]) are resharded with four
    per-batch AllToAll collectives so that each core ends up with the full
    1024 attn features for 1/8 of the t positions; the first three overlap
    the remaining compute.
  - Each core then applies the full Wproj to its t-slices (data-parallel), so
    no reduction collective is needed.

All matmuls run in float32r (fp32 stored, E8M11-rounded inputs, fp32
accumulate) which streams at full PE rate for moving dims >= 256.

Everything on-device is laid out "transposed" ([feature, t]) so that the
contraction dim of every matmul lands on SBUF partitions and no transposes
are needed anywhere except V (done on the PE with an identity matmul).

Softmax: scores are O(1) (inputs are unit-scale gaussians), so exp() without
max-subtraction is safe in fp32. The softmax denominator is produced by the
same matmul that computes attn@V via a ones-column appended to V; the final
divide is a DVE reciprocal + a GpSimd partition-broadcast + a DVE multiply,
applied straight out of PSUM.
"""

import numpy as np

import concourse.mybir as mybir
import concourse.tile as tile
from concourse import bacc
from concourse.bass_utils import run_bass_kernel_spmd

B, T, C, H, D = 4, 2048, 1024, 16, 64
NCORE = 8
HPC = H // NCORE  # 2 heads per core
BT = B * T
TCH = 512  # t-chunk width for qkv / scores free dim
NKT = T // 128  # 16 k-tiles per batch
NQC = T // TCH  # 4 q-chunks per batch

F32 = mybir.dt.float32
F32R = mybir.dt.float32r
EXP = mybir.ActivationFunctionType.Exp


def round_fp32r(a: np.ndarray) -> np.ndarray:
    """Round fp32 to E8M11 (fp32r) with round-to-nearest-even, as the HW does."""
    u = np.ascontiguousarray(a, dtype=np.float32).view(np.uint32)
    lsb = (u >> 12) & 1
    r = (u + 0x7FF + lsb) & 0xFFFFF000
    return r.view(np.float32)


def build_nc(sim_mode: bool = False, max_stage: int = 99):
    # sim_mode: skip collectives (TimelineSim is single-core) — timing study only
    # max_stage: emit only the first N stages (timing bisection in sim_mode)
    nc = bacc.Bacc("TRN2", target_bir_lowering=False, debug=False, num_devices=NCORE)

    xT = nc.dram_tensor("xT", [C, BT], F32R, kind="ExternalInput")
    wqkvT = nc.dram_tensor("wqkvT", [C, 3 * 128], F32R, kind="ExternalInput")
    wprojT = nc.dram_tensor("wprojT", [C, C], F32R, kind="ExternalInput")
    ident = nc.dram_tensor("ident", [128, 128], F32, kind="ExternalInput")
    tri = nc.dram_tensor("tri", [128, 128], F32R, kind="ExternalInput")
    yT = nc.dram_tensor("yT", [C, 2 * TCH], F32, kind="ExternalOutput")

    # AllToAll buffers, one per batch: [8 chunks, 128 feat (2 heads), 256 t]
    QW = T // NCORE  # 256: per-core t-slice of one batch
    a2a_in = [
        nc.dram_tensor(f"a2a_in{i}", [NCORE, 128, QW], F32R, kind="Internal")
        for i in range(B)
    ]
    a2a_out = [
        nc.dram_tensor(f"a2a_out{i}", [NCORE, 128, QW], F32R, kind="Internal")
        for i in range(B)
    ]
    groups = [list(range(NCORE))]

    with tile.TileContext(nc) as tc:
        with (
            tc.tile_pool(name="const", bufs=1) as constp,
            tc.tile_pool(name="xt", bufs=16) as xtp,
            tc.tile_pool(name="kt", bufs=2) as ktp,
            tc.tile_pool(name="qt", bufs=2) as qtp,
            tc.tile_pool(name="vaug", bufs=2) as vaugp,
            tc.tile_pool(name="vstage", bufs=4) as vstagep,
            tc.tile_pool(name="pt", bufs=6) as ptp,
            tc.tile_pool(name="rec", bufs=3) as recp,
            tc.tile_pool(name="bcast", bufs=3) as bcastp,
            tc.tile_pool(name="ofin", bufs=2) as ofinp,
            tc.tile_pool(name="recv", bufs=16) as recvp,
            tc.tile_pool(name="ystage", bufs=2) as ystagep,
            tc.tile_pool(name="psq", bufs=2, space="PSUM") as psq,
            tc.tile_pool(name="pss", bufs=2, space="PSUM") as pss,
            tc.tile_pool(name="pso", bufs=2, space="PSUM") as pso,
        ):
            # ---- constants ----
            # wqkv loads are interleaved with the first x chunk (see qkv_batch)
            wqkv_sb = constp.tile([128, C // 128, 3 * 128], F32R, tag="wqkv")
            wproj_sb = constp.tile([128, C // 128, C], F32R, tag="wproj")

            def load_wproj():
                # deferred: wproj is only needed by proj_quarter(0), far into the
                # kernel — keep it off the startup DMA critical path
                for ct in range(C // 128):
                    nc.sync.dma_start(
                        wproj_sb[:, ct], wprojT[ct * 128 : (ct + 1) * 128, :]
                    )
            ident_sb = constp.tile([128, 128], F32, tag="ident")
            nc.sync.dma_start(ident_sb[:], ident[:])
            tri_sb = constp.tile([128, 128], F32R, tag="tri")
            nc.sync.dma_start(tri_sb[:], tri[:])

            # Pre-zero score PSUM slots: diagonal tiles only write the causal
            # column range, and exp() reads the full (paired) range; stale
            # bits from uninitialized PSUM could be NaN/Inf otherwise.
            for _ in range(2):
                z = pss.tile([128, 2 * TCH], F32, tag="s")
                nc.vector.memset(z[:], 0.0)

            def qkv_batch(b):
                """Q^T,K^T: [128 (2 heads x 64d), 2048] f32r. V -> vaug tiles."""
                kt_t = ktp.tile([128, T], F32R, tag="kt")
                qt_t = qtp.tile([128, T], F32R, tag="qt")
                va_t = vaugp.tile([128, NKT, 130], F32R, tag="vaug")
                # ones columns at 64 and 129 of each [*, kt, :] slice: fill the
                # whole tile with 1.0; the V copies overwrite cols 0:64, 65:129
                nc.gpsimd.memset(va_t[:].bitcast(F32), 1.0)
                for tch in range(T // TCH):
                    t0 = b * T + tch * TCH
                    xts = []
                    for ct in range(C // 128):
                        if b == 0 and tch == 0:
                            # interleave weight-tile loads with the first x
                            # chunk so the first matmul chain starts early
                            nc.sync.dma_start(
                                wqkv_sb[:, ct], wqkvT[ct * 128 : (ct + 1) * 128, :]
                            )
                        xt_tile = xtp.tile([128, TCH], F32R, tag="xt")
                        nc.sync.dma_start(
                            xt_tile[:], xT[ct * 128 : (ct + 1) * 128, t0 : t0 + TCH]
                        )
                        xts.append(xt_tile)
                    for o in range(3):  # q, k, v feature blocks (128 each)
                        ps = psq.tile([128, TCH], F32, tag="q")
                        for ct in range(C // 128):
                            nc.tensor.matmul(
                                ps[:],
                                wqkv_sb[:, ct, o * 128 : (o + 1) * 128],
                                xts[ct][:],
                                start=(ct == 0),
                                stop=(ct == C // 128 - 1),
                            )
                        sl = slice(tch * TCH, (tch + 1) * TCH)
                        if o == 0:
                            nc.vector.tensor_copy(qt_t[:, sl], ps[:])
                        elif o == 1:
                            nc.vector.tensor_copy(kt_t[:, sl], ps[:])
                        else:
                            vs = vstagep.tile([128, TCH], F32, tag="vs")
                            nc.vector.tensor_copy(vs[:], ps[:])
                            for tt in range(TCH // 128):
                                kti = tch * (TCH // 128) + tt
                                psv = psq.tile([128, 128], F32, tag="q")
                                nc.tensor.transpose(
                                    psv[:],
                                    vs[:, tt * 128 : (tt + 1) * 128],
                                    ident_sb[:],
                                )
                                # [128 t, 128 d2] -> vaug cols {0:64, 65:129}
                                dst = va_t[:, kti].rearrange(
                                    "p (two s) -> p two s", s=65
                                )[:, :, 0:64]
                                nc.vector.tensor_copy(
                                    dst, psv[:].rearrange("p (two s) -> p two s", s=64)
                                )
                return qt_t, kt_t, va_t

            def attn_batch(b, qt_t, kt_t, va_t):
                ofin = [ofinp.tile([64, T], F32R, tag="ofin", name=f"ofin{hl}") for hl in range(HPC)]
                for qc in range(NQC):
                    ktmax = (qc + 1) * (TCH // 128)
                    psO = [pso.tile([65, TCH], F32, tag="o", name=f"psO{hl}") for hl in range(HPC)]
                    for ktp_i in range(ktmax // 2):
                        kts = [2 * ktp_i, 2 * ktp_i + 1]
                        colLo = [max(0, 128 * kt - TCH * qc) for kt in kts]
                        psS = [pss.tile([128, 2 * TCH], F32, tag="s", name=f"psS{hl}")
                               for hl in range(HPC)]
                        pt = [ptp.tile([128, 2 * TCH], F32R, tag="pt", name=f"pt{hl}")
                              for hl in range(HPC)]
                        # scores: the two heads' K=64 matmuls go to disjoint
                        # PE row groups (base partitions 0 / 64) and overlap
                        for i, kt in enumerate(kts):
                            for hl in range(HPC):
                                nc.tensor.matmul(
                                    psS[hl][:, TCH * i + colLo[i] : TCH * (i + 1)],
                                    kt_t[64 * hl : 64 * hl + 64,
                                         128 * kt : 128 * (kt + 1)],
                                    qt_t[64 * hl : 64 * hl + 64,
                                         TCH * qc + colLo[i] : TCH * (qc + 1)],
                                    start=True,
                                    stop=True,
                                )
                        for hl in range(HPC):
                            if colLo[0] == 0 and colLo[1] == 0:
                                nc.scalar.activation(
                                    pt[hl][:], psS[hl][:], EXP, scale=0.125
                                )
                            else:
                                # one strided op covering both halves from the
                                # smaller colLo; the extra columns in the
                                # second half are unused downstream
                                lo = min(colLo)
                                src = psS[hl][:].rearrange(
                                    "p (two x) -> p two x", two=2
                                )[:, :, lo:TCH]
                                dst = pt[hl][:].rearrange(
                                    "p (two x) -> p two x", two=2
                                )[:, :, lo:TCH]
                                nc.scalar.activation(dst, src, EXP, scale=0.125)
                        for hl in range(HPC):
                            for i, kt in enumerate(kts):
                                lo = colLo[i]
                                if kt >= 4 * qc:  # diagonal: mask boundary block
                                    nc.gpsimd.tensor_mul(
                                        pt[hl][:, TCH * i + lo : TCH * i + lo + 128],
                                        pt[hl][:, TCH * i + lo : TCH * i + lo + 128],
                                        tri_sb[:],
                                    )
                        for i, kt in enumerate(kts):
                            for hl in range(HPC):
                                nc.tensor.matmul(
                                    psO[hl][:, colLo[i] : TCH],
                                    va_t[:, kt, 65 * hl : 65 * (hl + 1)],
                                    pt[hl][:, TCH * i + colLo[i] : TCH * (i + 1)],
                                    start=(kt == 0),
                                    stop=(kt == ktmax - 1),
                                )
                    # normalize straight out of PSUM: denominator is psO row 64
                    for hl in range(HPC):
                        sl = slice(TCH * qc, TCH * (qc + 1))
                        rec = recp.tile([1, TCH], F32R, tag="rec")
                        with nc.allow_low_precision("fp32r softmax denominators"):
                            nc.vector.reciprocal(rec[:], psO[hl][64:65, :])
                        bc = bcastp.tile([64, TCH], F32R, tag="bc", name=f"bc{hl}")
                        nc.gpsimd.partition_broadcast(bc[:], rec[:])
                        nc.vector.tensor_mul(ofin[hl][:, sl], psO[hl][0:64, :], bc[:])
                        # ship to the AllToAll send buffers (2 chunks per qc)
                        for half in range(2):
                            j = 2 * qc + half
                            nc.sync.dma_start(
                                a2a_in[b][j, 64 * hl : 64 * hl + 64, :],
                                ofin[hl][:, TCH * qc + QW * half :
                                          TCH * qc + QW * (half + 1)],
                            )

            def proj_quarter(b):
                recvs = []
                for ct in range(C // 128):
                    r = recvp.tile([128, QW], F32R, tag="recv")
                    nc.sync.dma_start(r[:], a2a_out[b][ct])
                    recvs.append(r)
                for o in range(C // 128):
                    psY = pso.tile([128, QW], F32, tag="o", name=f"psY{o}")
                    for ct in range(C // 128):
                        nc.tensor.matmul(
                            psY[:],
                            wproj_sb[:, ct, o * 128 : (o + 1) * 128],
                            recvs[ct][:],
                            start=(ct == 0),
                            stop=(ct == C // 128 - 1),
                        )
                    ys = ystagep.tile([128, QW], F32, tag="ys")
                    nc.vector.tensor_copy(ys[:], psY[:])
                    nc.sync.dma_start(
                        yT[o * 128 : (o + 1) * 128, QW * b : QW * (b + 1)],
                        ys[:],
                    )

            def a2a(b):
                if sim_mode:
                    return
                nc.gpsimd.collective_compute(
                    "AllToAll",
                    mybir.AluOpType.bypass,
                    replica_groups=groups,
                    ins=[a2a_in[b][:]],
                    outs=[a2a_out[b][:]],
                )

            stage = 0
            for b in range(B):
                if stage >= max_stage:
                    break
                stage += 1
                tiles = qkv_batch(b)
                if stage >= max_stage:
                    break
                stage += 1
                attn_batch(b, *tiles)
                a2a(b)
                if b == 0:
                    load_wproj()
                if stage < max_stage:
                    stage += 1
                    proj_quarter(b)

    nc.compile()
    return nc


_NC_CACHE = None


def kernel(x: np.ndarray, Wqkv: np.ndarray, Wproj: np.ndarray) -> np.ndarray:
    global _NC_CACHE
    x = np.asarray(x, dtype=np.float32)
    Wqkv = np.asarray(Wqkv, dtype=np.float32)
    Wproj = np.asarray(Wproj, dtype=np.float32)

    xT = round_fp32r(x.reshape(BT, C).T)
    wprojT = round_fp32r(Wproj.T)
    ident = np.eye(128, dtype=np.float32)
    r = np.arange(128)
    tri = (r[:, None] <= r[None, :]).astype(np.float32)  # valid iff row <= col

    in_maps = []
    for c in range(NCORE):
        rows = slice(c * HPC * D, (c + 1) * HPC * D)  # 128 feature rows
        wq = Wqkv[0 * C :][rows]
        wk = Wqkv[1 * C :][rows]
        wv = Wqkv[2 * C :][rows]
        wqkvT_c = round_fp32r(np.concatenate([wq, wk, wv], axis=0).T)
        in_maps.append(
            {
                "xT": xT,
                "wqkvT": wqkvT_c,
                "wprojT": wprojT,
                "ident": ident,
                "tri": tri,
            }
        )

    if _NC_CACHE is None:
        _NC_CACHE = build_nc()
    res = run_bass_kernel_spmd(_NC_CACHE, in_maps, core_ids=list(range(NCORE)))

    # reassemble: core j returned yT_j [1024, 4*256]; quarter b holds the
    # t-slice [2048*b + 256*j, 2048*b + 256*(j+1)) of the full output
    QW = T // NCORE
    yT = np.empty((C, BT), dtype=np.float32)
    for j, r_ in enumerate(res.results):
        yTj = r_["yT"]
        for b in range(B):
            yT[:, T * b + QW * j : T * b + QW * (j + 1)] = (
                yTj[:, QW * b : QW * (b + 1)]
            )
    return np.ascontiguousarray(yT.T).reshape(B, T, C)


# revision 44
# speedup vs baseline: 129.1044x; 1.0753x over previous
"""Megatron-style MHA on 8 Trainium2 NeuronCores.

Problem: B=4, T=2048, C=1024, 16 heads, head_dim=64, causal attention, fp32.
  qkv = x @ Wqkv^T; attention per head; out = attn @ Wproj^T

Sharding (tensor-parallel over heads + AllToAll reshard):
  - Core c owns heads {2c, 2c+1}: computes Q/K/V (column-parallel Wqkv slice)
    and causal attention for those heads over all batches/positions.
  - Attention outputs (kept transposed: [feature, t]) are resharded with four
    per-batch AllToAll collectives so that each core ends up with the full
    1024 attn features for 1/8 of the t positions; the first three overlap
    the remaining compute.
  - Each core then applies the full Wproj to its t-slices (data-parallel), so
    no reduction collective is needed.

All matmuls run in float32r (fp32 stored, E8M11-rounded inputs, fp32
accumulate) which streams at full PE rate for moving dims >= 256.

Everything on-device is laid out "transposed" ([feature, t]) so that the
contraction dim of every matmul lands on SBUF partitions and no transposes
are needed anywhere except V (done on the PE with an identity matmul).

Softmax: scores are O(1) (inputs are unit-scale gaussians), so exp() without
max-subtraction is safe in fp32. The softmax denominator is produced by the
same matmul that computes attn@V via a ones-column appended to V; the final
divide is a DVE reciprocal + a GpSimd partition-broadcast + a DVE multiply,
applied straight out of PSUM.
"""

import numpy as np

import concourse.mybir as mybir
import concourse.tile as tile
from concourse import bacc
from concourse.bass_utils import run_bass_kernel_spmd

B, T, C, H, D = 4, 2048, 1024, 16, 64
NCORE = 8
HPC = H // NCORE  # 2 heads per core
BT = B * T
TCH = 512  # t-chunk width for qkv / scores free dim
NKT = T // 128  # 16 k-tiles per batch
NQC = T // TCH  # 4 q-chunks per batch

F32 = mybir.dt.float32
F32R = mybir.dt.float32r
EXP = mybir.ActivationFunctionType.Exp


def round_fp32r(a: np.ndarray) -> np.ndarray:
    """Round fp32 to E8M11 (fp32r) with round-to-nearest-even, as the HW does."""
    u = np.ascontiguousarray(a, dtype=np.float32).view(np.uint32)
    lsb = (u >> 12) & 1
    r = (u + 0x7FF + lsb) & 0xFFFFF000
    return r.view(np.float32)


def build_nc(sim_mode: bool = False, max_stage: int = 99):
    # sim_mode: skip collectives (TimelineSim is single-core) — timing study only
    # max_stage: emit only the first N stages (timing bisection in sim_mode)
    nc = bacc.Bacc("TRN2", target_bir_lowering=False, debug=False, num_devices=NCORE)

    xT = nc.dram_tensor("xT", [C, BT], F32R, kind="ExternalInput")
    wqkvT = nc.dram_tensor("wqkvT", [C, 3 * 128], F32R, kind="ExternalInput")
    wprojT = nc.dram_tensor("wprojT", [C, C], F32R, kind="ExternalInput")
    ident = nc.dram_tensor("ident", [128, 128], F32, kind="ExternalInput")
    tri = nc.dram_tensor("tri", [128, 128], F32R, kind="ExternalInput")
    tri3 = nc.dram_tensor("tri3", [128, 256], F32R, kind="ExternalInput")
    yT = nc.dram_tensor("yT", [C, 2 * TCH], F32, kind="ExternalOutput")

    # AllToAll buffers, one per batch: [8 chunks, 128 feat (2 heads), 256 t]
    QW = T // NCORE  # 256: per-core t-slice of one batch
    a2a_in = [
        nc.dram_tensor(f"a2a_in{i}", [NCORE, 128, QW], F32R, kind="Internal")
        for i in range(B)
    ]
    a2a_out = [
        nc.dram_tensor(f"a2a_out{i}", [NCORE, 128, QW], F32R, kind="Internal")
        for i in range(B)
    ]
    groups = [list(range(NCORE))]

    with tile.TileContext(nc) as tc:
        with (
            tc.tile_pool(name="const", bufs=1) as constp,
            tc.tile_pool(name="xt", bufs=16) as xtp,
            tc.tile_pool(name="kt", bufs=2) as ktp,
            tc.tile_pool(name="qt", bufs=2) as qtp,
            tc.tile_pool(name="vaug", bufs=2) as vaugp,
            tc.tile_pool(name="vstage", bufs=4) as vstagep,
            tc.tile_pool(name="pt", bufs=6) as ptp,
            tc.tile_pool(name="rec", bufs=3) as recp,
            tc.tile_pool(name="bcast", bufs=3) as bcastp,
            tc.tile_pool(name="ofin", bufs=2) as ofinp,
            tc.tile_pool(name="recv", bufs=16) as recvp,
            tc.tile_pool(name="ystage", bufs=2) as ystagep,
            tc.tile_pool(name="psq", bufs=2, space="PSUM") as psq,
            tc.tile_pool(name="pss", bufs=2, space="PSUM") as pss,
            tc.tile_pool(name="pso", bufs=2, space="PSUM") as pso,
        ):
            # ---- constants ----
            # wqkv loads are interleaved with the first x chunk (see qkv_batch)
            wqkv_sb = constp.tile([128, C // 128, 3 * 128], F32R, tag="wqkv")
            wproj_sb = constp.tile([128, C // 128, C], F32R, tag="wproj")

            def load_wproj():
                # deferred: wproj is only needed by proj_quarter(0), far into the
                # kernel — keep it off the startup DMA critical path
                for ct in range(C // 128):
                    nc.sync.dma_start(
                        wproj_sb[:, ct], wprojT[ct * 128 : (ct + 1) * 128, :]
                    )
            ident_sb = constp.tile([128, 128], F32, tag="ident")
            nc.sync.dma_start(ident_sb[:], ident[:])
            tri_sb = constp.tile([128, 128], F32R, tag="tri")
            nc.sync.dma_start(tri_sb[:], tri[:])
            tri3_sb = constp.tile([128, 256], F32R, tag="tri3")
            nc.sync.dma_start(tri3_sb[:], tri3[:])

            # Pre-zero score PSUM slots: diagonal tiles only write the causal
            # column range, and exp() reads the full (paired) range; stale
            # bits from uninitialized PSUM could be NaN/Inf otherwise.
            for _ in range(2):
                z = pss.tile([128, 2 * TCH], F32, tag="s")
                nc.vector.memset(z[:], 0.0)

            def qkv_batch(b):
                """Q^T,K^T: [128 (2 heads x 64d), 2048] f32r. V -> vaug tiles."""
                kt_t = ktp.tile([128, T], F32R, tag="kt")
                qt_t = qtp.tile([128, T], F32R, tag="qt")
                va_t = vaugp.tile([128, NKT, 130], F32R, tag="vaug")
                # ones columns at 64 and 129 of each [*, kt, :] slice: fill the
                # whole tile with 1.0; the V copies overwrite cols 0:64, 65:129
                nc.gpsimd.memset(va_t[:].bitcast(F32), 1.0)
                for tch in range(T // TCH):
                    t0 = b * T + tch * TCH
                    xts = []
                    for ct in range(C // 128):
                        if b == 0 and tch == 0:
                            # interleave weight-tile loads with the first x
                            # chunk so the first matmul chain starts early
                            nc.sync.dma_start(
                                wqkv_sb[:, ct], wqkvT[ct * 128 : (ct + 1) * 128, :]
                            )
                        xt_tile = xtp.tile([128, TCH], F32R, tag="xt")
                        nc.sync.dma_start(
                            xt_tile[:], xT[ct * 128 : (ct + 1) * 128, t0 : t0 + TCH]
                        )
                        xts.append(xt_tile)
                    for o in range(3):  # q, k, v feature blocks (128 each)
                        ps = psq.tile([128, TCH], F32, tag="q")
                        for ct in range(C // 128):
                            nc.tensor.matmul(
                                ps[:],
                                wqkv_sb[:, ct, o * 128 : (o + 1) * 128],
                                xts[ct][:],
                                start=(ct == 0),
                                stop=(ct == C // 128 - 1),
                            )
                        sl = slice(tch * TCH, (tch + 1) * TCH)
                        if o == 0:
                            nc.vector.tensor_copy(qt_t[:, sl], ps[:])
                        elif o == 1:
                            nc.vector.tensor_copy(kt_t[:, sl], ps[:])
                        else:
                            vs = vstagep.tile([128, TCH], F32, tag="vs")
                            nc.vector.tensor_copy(vs[:], ps[:])
                            for tt in range(TCH // 128):
                                kti = tch * (TCH // 128) + tt
                                psv = pso.tile([128, 128], F32, tag="o", name="psv")
                                nc.tensor.transpose(
                                    psv[:],
                                    vs[:, tt * 128 : (tt + 1) * 128],
                                    ident_sb[:],
                                )
                                # [128 t, 128 d2] -> vaug cols {0:64, 65:129}
                                dst = va_t[:, kti].rearrange(
                                    "p (two s) -> p two s", s=65
                                )[:, :, 0:64]
                                nc.vector.tensor_copy(
                                    dst, psv[:].rearrange("p (two s) -> p two s", s=64)
                                )
                return qt_t, kt_t, va_t

            def attn_batch(b, qt_t, kt_t, va_t):
                ofin = [ofinp.tile([64, T], F32R, tag="ofin", name=f"ofin{hl}") for hl in range(HPC)]
                for qc in range(NQC):
                    ktmax = (qc + 1) * (TCH // 128)
                    psO = [pso.tile([65, TCH], F32, tag="o", name=f"psO{hl}") for hl in range(HPC)]
                    for ktp_i in range(ktmax // 2):
                        kts = [2 * ktp_i, 2 * ktp_i + 1]
                        trueLo = [max(0, 128 * kt - TCH * qc) for kt in kts]
                        colLo = [min(lo, 256) for lo in trueLo]
                        psS = [pss.tile([128, 2 * TCH], F32, tag="s", name=f"psS{hl}")
                               for hl in range(HPC)]
                        pt = [ptp.tile([128, 2 * TCH], F32R, tag="pt", name=f"pt{hl}")
                              for hl in range(HPC)]
                        # scores: the two heads' K=64 matmuls go to disjoint
                        # PE row groups (base partitions 0 / 64) and overlap
                        for i, kt in enumerate(kts):
                            for hl in range(HPC):
                                nc.tensor.matmul(
                                    psS[hl][:, TCH * i + colLo[i] : TCH * (i + 1)],
                                    kt_t[64 * hl : 64 * hl + 64,
                                         128 * kt : 128 * (kt + 1)],
                                    qt_t[64 * hl : 64 * hl + 64,
                                         TCH * qc + colLo[i] : TCH * (qc + 1)],
                                    start=True,
                                    stop=True,
                                )
                        for hl in range(HPC):
                            if colLo[0] == 0 and colLo[1] == 0:
                                nc.scalar.activation(
                                    pt[hl][:], psS[hl][:], EXP, scale=0.125
                                )
                            else:
                                # one strided op covering both halves from the
                                # smaller colLo; the extra columns in the
                                # second half are unused downstream
                                lo = min(colLo)
                                src = psS[hl][:].rearrange(
                                    "p (two x) -> p two x", two=2
                                )[:, :, lo:TCH]
                                dst = pt[hl][:].rearrange(
                                    "p (two x) -> p two x", two=2
                                )[:, :, lo:TCH]
                                nc.scalar.activation(dst, src, EXP, scale=0.125)
                        for hl in range(HPC):
                            for i, kt in enumerate(kts):
                                lo = colLo[i]
                                if kt < 4 * qc:
                                    continue  # fully below the diagonal
                                if trueLo[i] > lo:
                                    # capped slice: zero [lo, trueLo) + triangle
                                    nc.vector.tensor_mul(
                                        pt[hl][:, TCH * i + lo : TCH * i + lo + 256],
                                        pt[hl][:, TCH * i + lo : TCH * i + lo + 256],
                                        tri3_sb[:],
                                    )
                                else:
                                    nc.vector.tensor_mul(
                                        pt[hl][:, TCH * i + lo : TCH * i + lo + 128],
                                        pt[hl][:, TCH * i + lo : TCH * i + lo + 128],
                                        tri_sb[:],
                                    )
                        for i, kt in enumerate(kts):
                            for hl in range(HPC):
                                nc.tensor.matmul(
                                    psO[hl][:, colLo[i] : TCH],
                                    va_t[:, kt, 65 * hl : 65 * (hl + 1)],
                                    pt[hl][:, TCH * i + colLo[i] : TCH * (i + 1)],
                                    start=(kt == 0),
                                    stop=(kt == ktmax - 1),
                                )
                    # normalize straight out of PSUM: denominator is psO row 64
                    for hl in range(HPC):
                        sl = slice(TCH * qc, TCH * (qc + 1))
                        rec = recp.tile([1, TCH], F32R, tag="rec")
                        with nc.allow_low_precision("fp32r softmax denominators"):
                            nc.vector.reciprocal(rec[:], psO[hl][64:65, :])
                        bc = bcastp.tile([64, TCH], F32R, tag="bc", name=f"bc{hl}")
                        nc.gpsimd.partition_broadcast(bc[:], rec[:])
                        nc.vector.tensor_mul(ofin[hl][:, sl], psO[hl][0:64, :], bc[:])
                        # ship to the AllToAll send buffers (2 chunks per qc)
                        for half in range(2):
                            j = 2 * qc + half
                            nc.sync.dma_start(
                                a2a_in[b][j, 64 * hl : 64 * hl + 64, :],
                                ofin[hl][:, TCH * qc + QW * half :
                                          TCH * qc + QW * (half + 1)],
                            )

            def proj_quarter(b):
                recvs = []
                for ct in range(C // 128):
                    r = recvp.tile([128, QW], F32R, tag="recv")
                    nc.sync.dma_start(r[:], a2a_out[b][ct])
                    recvs.append(r)
                for o in range(C // 128):
                    psY = psq.tile([128, QW], F32, tag="q", name=f"psY{o}")
                    for ct in range(C // 128):
                        nc.tensor.matmul(
                            psY[:],
                            wproj_sb[:, ct, o * 128 : (o + 1) * 128],
                            recvs[ct][:],
                            start=(ct == 0),
                            stop=(ct == C // 128 - 1),
                        )
                    ys = ystagep.tile([128, QW], F32, tag="ys")
                    nc.vector.tensor_copy(ys[:], psY[:])
                    nc.sync.dma_start(
                        yT[o * 128 : (o + 1) * 128, QW * b : QW * (b + 1)],
                        ys[:],
                    )

            def a2a(b):
                if sim_mode:
                    return
                nc.gpsimd.collective_compute(
                    "AllToAll",
                    mybir.AluOpType.bypass,
                    replica_groups=groups,
                    ins=[a2a_in[b][:]],
                    outs=[a2a_out[b][:]],
                )

            stage = 0
            for b in range(B):
                if stage >= max_stage:
                    break
                stage += 1
                tiles = qkv_batch(b)
                if stage >= max_stage:
                    break
                stage += 1
                attn_batch(b, *tiles)
                a2a(b)
                if b == 0:
                    load_wproj()
                if stage < max_stage:
                    stage += 1
                    proj_quarter(b)

    nc.compile()
    return nc


_NC_CACHE = None


def kernel(x: np.ndarray, Wqkv: np.ndarray, Wproj: np.ndarray) -> np.ndarray:
    global _NC_CACHE
    x = np.asarray(x, dtype=np.float32)
    Wqkv = np.asarray(Wqkv, dtype=np.float32)
    Wproj = np.asarray(Wproj, dtype=np.float32)

    xT = round_fp32r(x.reshape(BT, C).T)
    wprojT = round_fp32r(Wproj.T)
    ident = np.eye(128, dtype=np.float32)
    r = np.arange(128)
    tri = (r[:, None] <= r[None, :]).astype(np.float32)  # valid iff row <= col
    tri3 = np.concatenate(
        [np.zeros((128, 128), np.float32), tri], axis=1
    )  # wide mask for the N-capped deepest diagonal slice

    in_maps = []
    for c in range(NCORE):
        rows = slice(c * HPC * D, (c + 1) * HPC * D)  # 128 feature rows
        wq = Wqkv[0 * C :][rows]
        wk = Wqkv[1 * C :][rows]
        wv = Wqkv[2 * C :][rows]
        wqkvT_c = round_fp32r(np.concatenate([wq, wk, wv], axis=0).T)
        in_maps.append(
            {
                "xT": xT,
                "wqkvT": wqkvT_c,
                "wprojT": wprojT,
                "ident": ident,
                "tri": tri,
                "tri3": tri3,
            }
        )

    if _NC_CACHE is None:
        _NC_CACHE = build_nc()
    res = run_bass_kernel_spmd(_NC_CACHE, in_maps, core_ids=list(range(NCORE)))

    # reassemble: core j returned yT_j [1024, 4*256]; quarter b holds the
    # t-slice [2048*b + 256*j, 2048*b + 256*(j+1)) of the full output
    QW = T // NCORE
    yT = np.empty((C, BT), dtype=np.float32)
    for j, r_ in enumerate(res.results):
        yTj = r_["yT"]
        for b in range(B):
            yT[:, T * b + QW * j : T * b + QW * (j + 1)] = (
                yTj[:, QW * b : QW * (b + 1)]
            )
    return np.ascontiguousarray(yT.T).reshape(B, T, C)
